# revision 10
# baseline (speedup 1.0000x reference)
"""Dot-product attention (B=2, H=16, S=2048, D=64, int32 mask) on 8 TRN2 cores.

Sharding: 32 (batch, head) pairs -> 4 heads per core; cores 0-3 take batch 0,
cores 4-7 take batch 1, so each core needs only its batch's (S, S) mask.

Per-core algorithm (scores kept TRANSPOSED, k on partitions, so the big P
matrix never needs an on-device transpose):
  mm1 : S^T[kc] (128 x 1024) = K^T_chunk.T @ Q^T         (fp16 in, fp32 PSUM)
  exp : P^T = exp(S^T / 8 - 4) on ScalarE, fp16 out      (bias -4 cancels in
        the softmax ratio; scores are bounded so no row-max pass is needed)
  mask: P^T *= mask^T (fp16, DVE 2x mode; mask resident in SBUF, reused by
        all 4 heads)
  mm2 : acc^T[65, 1024] += [V | 1]_chunk.T @ P^T         (fp16, accumulated
        over the 16 k-chunks; the ones column makes row 64 the softmax
        denominator for free)
  out : raw acc^T tiles stream back to DRAM; the final divide-by-denominator
        and [d, q] -> [q, d] transpose are O(S*D) and run on the host.

mm2 is issued LAG chunks behind mm1 so the PE's strict FIFO never waits on
the exp->mask chain; this keeps PE duty high enough that the HAM clock gate
stays at 2.4 GHz.
"""

import contextlib
import ctypes
import sys

if "/opt/trn_rl_repo" not in sys.path:
    sys.path.insert(0, "/opt/trn_rl_repo")

import numpy as np

B, H, S, D = 2, 16, 2048, 64
NCORES = 8
HPC = (B * H) // NCORES  # heads per core = 4
KC = 128                 # k-chunk (partition) size
NKC = S // KC            # 16 k-chunks
QH = 1024                # q block size
NQH = S // QH            # 2 q blocks
MMF = 512                # per-matmul moving free dim (one PSUM bank)

_AXON_SO = "/opt/axon/libaxon_pjrt.so"


@contextlib.contextmanager
def _ntff_capture(output_dir, device_ids=None):
    """Drive libaxon_pjrt's NRT-profile ABI directly (antenv.axon_hooks is
    absent in this image, so run_bass_kernel_spmd(trace=True) can't)."""
    import jax

    lib = ctypes.CDLL(_AXON_SO)
    lib.axon_start_nrt_profile.argtypes = [
        ctypes.POINTER(ctypes.c_int64),
        ctypes.c_size_t,
    ]
    lib.axon_start_nrt_profile.restype = ctypes.c_int64
    lib.axon_stop_nrt_profile.argtypes = [ctypes.c_char_p]
    lib.axon_stop_nrt_profile.restype = ctypes.c_int64

    jax.devices()
    if device_ids:
        ids = (ctypes.c_int64 * len(device_ids))(*device_ids)
        rc = lib.axon_start_nrt_profile(ids, len(device_ids))
    else:
        rc = lib.axon_start_nrt_profile(None, 0)
    if rc != 0:
        raise RuntimeError(f"axon_start_nrt_profile rc={rc}")
    try:
        yield
    finally:
        n = lib.axon_stop_nrt_profile(str(output_dir).encode())
        print(f"ntff_capture: {n} file(s) -> {output_dir}", file=sys.stderr)


def build_nc(hpc=HPC, s=S, d=D, qh=QH):
    import concourse.bacc as bacc
    import concourse.tile as tile
    from concourse import mybir

    f32 = mybir.dt.float32
    f16 = mybir.dt.float16
    Exp = mybir.ActivationFunctionType.Exp

    nkc = s // KC
    nqh = s // qh
    dx = d + 1

    nc = bacc.Bacc("TRN2", target_bir_lowering=False, debug=False,
                   num_devices=NCORES)
    qT = nc.dram_tensor("qT", [hpc, d, s], f16, kind="ExternalInput").ap()
    kT = nc.dram_tensor("kT", [hpc, d, s], f16, kind="ExternalInput").ap()
    vx = nc.dram_tensor("vx", [hpc, s, dx], f16, kind="ExternalInput").ap()
    mT = nc.dram_tensor("mT", [s, s], f16, kind="ExternalInput").ap()
    oT = nc.dram_tensor("oT", [hpc, nqh, dx, qh], f32,
                        kind="ExternalOutput").ap()

    with tile.TileContext(nc) as tc:
        with (
            tc.tile_pool(name="const", bufs=1) as const,
            tc.tile_pool(name="headio", bufs=2) as headio,
            tc.tile_pool(name="pexp", bufs=3) as pexp,
            tc.tile_pool(name="pmask", bufs=4) as pmask,
            tc.tile_pool(name="accsb", bufs=2) as accsb,
            tc.tile_pool(name="pchunk", bufs=2, space="PSUM") as pchunk,
            tc.tile_pool(name="pacc", bufs=2, space="PSUM") as pacc,
        ):
            nbias = const.tile([128, 1], f32, tag="nbias")
            nc.vector.memset(nbias, -4.0)

            # Head-0 inputs issue before the 8.4MB mask burst so the first
            # matmul isn't starved behind the mask DMAs.
            kT0 = headio.tile([d, s], f16, tag="kT")
            nc.sync.dma_start(out=kT0, in_=kT[0])
            qT0 = headio.tile([d, s], f16, tag="qT")
            nc.sync.dma_start(out=qT0, in_=qT[0])
            vx0 = headio.tile([128, nkc, dx], f16, tag="vx")
            nc.sync.dma_start(out=vx0,
                              in_=vx[0].rearrange("(n p) x -> p n x", p=128))

            # Mask chunks as separate tiles so each consumer only waits on
            # its own chunk's DMA (SWDGE queue, distinct from head IO).
            mt_tiles = []
            for kc in range(nkc):
                t = const.tile([128, s], f16, tag=f"mT{kc}")
                nc.gpsimd.dma_start(out=t, in_=mT[kc * KC:(kc + 1) * KC, :])
                mt_tiles.append(t)

            LAG = 2  # mm2 trails mm1 by LAG chunks in the PE FIFO

            for h in range(hpc):
                if h == 0:
                    kT_sb, qT_sb, vx_sb = kT0, qT0, vx0
                else:
                    kT_sb = headio.tile([d, s], f16, tag="kT")
                    nc.sync.dma_start(out=kT_sb, in_=kT[h])
                    qT_sb = headio.tile([d, s], f16, tag="qT")
                    nc.sync.dma_start(out=qT_sb, in_=qT[h])
                    vx_sb = headio.tile([128, nkc, dx], f16, tag="vx")
                    nc.sync.dma_start(
                        out=vx_sb,
                        in_=vx[h].rearrange("(n p) x -> p n x", p=128))

                for q0 in range(nqh):
                    acc = pacc.tile([dx, qh], f32, tag="acc")
                    pms = {}

                    def mm2(kc):
                        for j in range(qh // MMF):
                            nc.tensor.matmul(
                                acc[:, j * MMF:(j + 1) * MMF],
                                lhsT=vx_sb[:, kc, :],
                                rhs=pms[kc][:, j * MMF:(j + 1) * MMF],
                                start=(kc == 0), stop=(kc == nkc - 1),
                                skip_group_check=True,
                            )
                        del pms[kc]

                    for kc in range(nkc):
                        chunk = pchunk.tile([128, qh], f32, tag="chunk")
                        for j in range(qh // MMF):
                            nc.tensor.matmul(
                                chunk[:, j * MMF:(j + 1) * MMF],
                                lhsT=kT_sb[:, kc * KC:(kc + 1) * KC],
                                rhs=qT_sb[:, q0 * qh + j * MMF:
                                          q0 * qh + (j + 1) * MMF],
                                start=True, stop=True,
                            )
                        pt = pexp.tile([128, qh], f16, tag="pt")
                        nc.scalar.activation(pt, chunk, Exp,
                                             bias=nbias, scale=0.125)
                        pm = pmask.tile([128, qh], f16, tag="pm")
                        nc.vector.tensor_mul(
                            pm, pt, mt_tiles[kc][:, q0 * qh:(q0 + 1) * qh])
                        pms[kc] = pm
                        if kc >= LAG:
                            mm2(kc - LAG)
                    for kc in range(nkc - LAG, nkc):
                        mm2(kc)
                    acc_sb = accsb.tile([dx, qh], f32, tag="accsb")
                    nc.vector.tensor_copy(acc_sb, acc)
                    nc.sync.dma_start(out=oT[h, q0], in_=acc_sb)
    nc.compile()
    return nc


def _shard_inputs(q, k, v, mask):
    """Host-side prep: per-core transposed fp16 Q/K, fp16 [V|1], fp16
    transposed mask (computed once per batch, shared by 4 cores)."""
    q = np.asarray(q, dtype=np.float32)
    k = np.asarray(k, dtype=np.float32)
    v = np.asarray(v, dtype=np.float32)
    mask = np.asarray(mask)

    maskT16 = [
        np.ascontiguousarray(mask[b, 0].T).astype(np.float16) for b in range(B)
    ]
    ones = np.ones((HPC, S, 1), dtype=np.float16)
    in_maps = []
    for c in range(NCORES):
        b = c // (NCORES // B)
        h0 = (c % (NCORES // B)) * HPC
        qh_ = q[b, h0:h0 + HPC]
        kh = k[b, h0:h0 + HPC]
        vh = v[b, h0:h0 + HPC]
        in_maps.append({
            "qT": np.ascontiguousarray(qh_.transpose(0, 2, 1)).astype(np.float16),
            "kT": np.ascontiguousarray(kh.transpose(0, 2, 1)).astype(np.float16),
            "vx": np.concatenate([vh.astype(np.float16), ones], axis=-1),
            "mT": maskT16[b],
        })
    return in_maps


_CACHED_NC = None


def kernel(q, k, v, mask, _profile_dir=None):
    global _CACHED_NC
    from concourse.bass_utils import run_bass_kernel_spmd

    if _CACHED_NC is None:
        _CACHED_NC = build_nc()
    nc = _CACHED_NC

    in_maps = _shard_inputs(q, k, v, mask)
    core_ids = list(range(NCORES))
    if _profile_dir is not None:
        with _ntff_capture(_profile_dir, [0]):
            res = run_bass_kernel_spmd(nc, in_maps, core_ids)
    else:
        res = run_bass_kernel_spmd(nc, in_maps, core_ids)

    out = np.empty((B, H, S, D), dtype=np.float32)
    for c in range(NCORES):
        b = c // (NCORES // B)
        h0 = (c % (NCORES // B)) * HPC
        a = res.results[c]["oT"]  # [HPC, NQH, D+1, QH]
        o = a[:, :, :D, :] / a[:, :, D:D + 1, :]
        # [hpc, nqh, d, qh] -> [hpc, nqh, qh, d] -> [hpc, S, D]
        out[b, h0:h0 + HPC] = o.transpose(0, 1, 3, 2).reshape(HPC, S, D)
    return out
